# revision 1
# baseline (speedup 1.0000x reference)
"""Trainium2 Bass kernel for AdditiveLowRankPairwise.

scores[b,t,s] = sum_r iw[r]*silu(pt[b,t,r]*ps[b,s,r]) + tl[b,t] + sl[b,s] + bias
  pt = target_val @ Wt.T   [B,T,R]
  ps = source_val @ Ws.T   [B,S,R]
  tl = pt @ wt_out         [B,T]
  sl = ps @ ws_out         [B,S]

B=2, T=S=1024, D=512, R=64.  8 cores: core c handles b=c//4, t-rows
[(c%4)*256, (c%4+1)*256).  Per core:
  - ps2 [128,1024] (r-duplicated on partition halves) is produced directly in
    PSUM by the projection matmuls using a host-duplicated stationary
    wsT2=[WsT|WsT]; it stays resident in PSUM for the whole kernel.
  - per pair p of t-rows (t_top=tb*128+p, t_bot=tb*128+64+p):
      one ACT instruction computes silu(ps2[q,s]*pt2[q,p]) via the
      per-partition scale operand (PSUM source), f32 output;
      PE matmul (float32r: full-rate, ~fp32 precision) with a
      2-one-hot-column stationary (slice of a host-built [128,192] matrix)
      accumulates the iw-weighted partition-reduction into score psum rows
      {p, 64+p}.
  - sl broadcast folded in via psum-initializing matmul (ws_out replicated
    stationary); tl+bias added in the PSUM->SBUF fixup (per-partition bias).

loop_n>0 wraps the body in an on-device For_i loop (wall-clock-delta timing).
"""

import numpy as np

B, T, S, D, R = 2, 1024, 1024, 512, 64
TBLK = 256          # t-rows per core
NCORES = 8
GRP = 8             # pairs per activation batch (dve_prod variant)
VARIANT = "act_fused"
_ACT_NAME = "Silu"  # sim override: CoreSim lacks Silu; tests may set "Sigmoid"

_compiled = {}


def _build_nc(variant=VARIANT, loop_n=0):
    import concourse.mybir as mybir
    import concourse.tile as tile
    from concourse import bacc

    f32 = mybir.dt.float32
    f32r = mybir.dt.float32r
    AF = mybir.ActivationFunctionType
    AF_SILU = getattr(AF, _ACT_NAME)
    ET = mybir.EngineType

    nc = bacc.Bacc("TRN2", target_bir_lowering=False, debug=False)

    tvT = nc.dram_tensor("tvT", [D, TBLK], f32r, kind="ExternalInput")
    svT = nc.dram_tensor("svT", [D, S], f32r, kind="ExternalInput")
    wtT = nc.dram_tensor("wtT", [D, R], f32r, kind="ExternalInput")
    wsT2 = nc.dram_tensor("wsT2", [D, 128], f32r, kind="ExternalInput")
    wtb_col = nc.dram_tensor("wtb_col", [R + 1, 1], f32r, kind="ExternalInput")
    ws_rep256 = nc.dram_tensor("ws_rep256", [R, TBLK], f32r,
                               kind="ExternalInput")
    big = nc.dram_tensor("big", [128, 192], f32r, kind="ExternalInput")
    bias_row = nc.dram_tensor("bias_row", [1, TBLK], f32r,
                              kind="ExternalInput")
    ones_row = nc.dram_tensor("ones_row", [1, S], f32r, kind="ExternalInput")
    out = nc.dram_tensor("out", [TBLK, S], f32, kind="ExternalOutput")

    with tile.TileContext(nc) as tc:
        with (
            tc.tile_pool(name="const", bufs=1) as cpool,
            tc.tile_pool(name="ptb", bufs=2) as ptbpool,
            tc.tile_pool(name="prod", bufs=2) as prodpool,
            tc.tile_pool(name="actb",
                         bufs=(4 if variant == "act_fused" else 2)) as actpool,
            tc.tile_pool(name="ps2_psum", bufs=1, space="PSUM") as ps2pool,
            tc.tile_pool(name="pt_psum", bufs=1, space="PSUM") as ptpool,
            tc.tile_pool(name="tl_psum", bufs=1, space="PSUM") as tlpool,
            tc.tile_pool(name="score_psum", bufs=2, space="PSUM") as spool,
            tc.tile_pool(name="outsb", bufs=2) as outpool,
        ):
            def emit_body():
                wtT_sb = cpool.tile([128, 4 * R], f32r, tag="wtT_sb")
                wsT2_sb = cpool.tile([128, 4 * 128], f32r, tag="wsT2_sb")
                wtb_sb = cpool.tile([R + 1, 1], f32r, tag="wtb_sb")
                slt_stat = cpool.tile([R + 1, TBLK], f32r, tag="slt_stat")
                big_sb = cpool.tile([128, 192], f32r, tag="big_sb")
                tv_sb = cpool.tile([128, 4 * TBLK], f32r, tag="tv_sb")
                sv_k = [cpool.tile([128, S], f32r, tag=f"sv_{k}",
                                   name=f"sv_{k}")
                        for k in range(4)]
                ps2_sb = cpool.tile([128, S], f32, tag="ps2_sb")
                psl = cpool.tile([R + 1, S], f32r, tag="psl")
                pt_sb = cpool.tile([R + 1, TBLK], f32r, tag="pt_sb")

                for k in range(4):
                    nc.sync.dma_start(out=sv_k[k][:],
                                      in_=svT[k * 128:(k + 1) * 128, :])
                    nc.sync.dma_start(out=wtT_sb[:, k * R:(k + 1) * R],
                                      in_=wtT[k * 128:(k + 1) * 128, :])
                    nc.sync.dma_start(out=wsT2_sb[:, k * 128:(k + 1) * 128],
                                      in_=wsT2[k * 128:(k + 1) * 128, :])
                    nc.sync.dma_start(out=tv_sb[:, k * TBLK:(k + 1) * TBLK],
                                      in_=tvT[k * 128:(k + 1) * 128, :])
                nc.sync.dma_start(out=wtb_sb[:], in_=wtb_col[:])
                nc.sync.dma_start(out=slt_stat[0:R, :], in_=ws_rep256[:])
                nc.sync.dma_start(out=big_sb[:], in_=big[:])
                nc.sync.dma_start(out=pt_sb[R:R + 1, :], in_=bias_row[:])

                # ---- projections on PE (float32r, full rate) ----
                # ps2 directly in PSUM, r duplicated on partition halves via
                # the host-duplicated stationary wsT2.
                ps2 = ps2pool.tile([128, S], f32, tag="ps2")
                for kc in range(4):
                    for nh in range(2):
                        nc.tensor.matmul(
                            ps2[:, nh * 512:(nh + 1) * 512],
                            (wsT2_sb[:, kc * 128:(kc + 1) * 128]),
                            (sv_k[kc][:, nh * 512:(nh + 1) * 512]),
                            start=(kc == 0), stop=(kc == 3))
                pt_ps = ptpool.tile([R, TBLK], f32, tag="pt_ps")
                for kc in range(4):
                    nc.tensor.matmul(
                        pt_ps[:],
                        (wtT_sb[:, kc * R:(kc + 1) * R]),
                        (tv_sb[:, kc * TBLK:(kc + 1) * TBLK]),
                        start=(kc == 0), stop=(kc == 3))
                # SBUF copies: full duplicated ps2 (ACT input), psl (rows
                # 0:64 = ps + a ones row 64) for the psum-init matmul, pt
                nc.vector.tensor_copy(ps2_sb[:], ps2[:])
                nc.vector.tensor_copy(psl[0:R, :], ps2[0:R, :])
                nc.sync.dma_start(out=psl[R:R + 1, :], in_=ones_row[:])
                nc.vector.tensor_copy(pt_sb[0:R, :], pt_ps[:])

                # tl+bias row: one matmul over [65,(pt;bias_row)] -> [1, 256]
                tl_ps = tlpool.tile([1, TBLK], f32, tag="tl_ps")
                nc.tensor.matmul(tl_ps[:], (wtb_sb[:]), (pt_sb[:]),
                                 start=True, stop=True)
                nc.vector.tensor_copy(slt_stat[R:R + 1, :], tl_ps[:])

                for tb in range(2):
                    ptb2 = ptbpool.tile([128, R], f32, tag="ptb2")
                    nc.vector.tensor_copy(ptb2[0:R, :],
                                          pt_sb[0:R, tb * 128: tb * 128 + R])
                    nc.vector.tensor_copy(
                        ptb2[R:128, :],
                        pt_sb[0:R, tb * 128 + R: tb * 128 + 128])

                    score_ps = spool.tile([128, S], f32, tag="score_ps")
                    # init psum with sl[s] + tl[t] + bias in one matmul
                    for nh in range(2):
                        nc.tensor.matmul(
                            score_ps[:, nh * 512:(nh + 1) * 512],
                            (slt_stat[:, tb * 128:(tb + 1) * 128]),
                            (psl[:, nh * 512: nh * 512 + 512]),
                            start=True, stop=False)

                    if variant == "act_fused":
                        for p in range(64):
                            actb = actpool.tile([128, S], f32r, tag="actb")
                            nc.scalar.activation(actb[:], ps2_sb[:], AF_SILU,
                                                 scale=ptb2[:, p:p + 1])
                            last = (p == 63)
                            for nh in range(2):
                                nc.tensor.matmul(
                                    score_ps[:, nh * 512:(nh + 1) * 512],
                                    (big_sb[:, 63 - p: 63 - p + 128]),
                                    (actb[:, nh * 512: nh * 512 + 512]),
                                    start=False, stop=last)
                    else:  # dve_prod
                        npair = 64 // GRP
                        for g in range(GRP):
                            prod = prodpool.tile([128, npair * S], f32,
                                                 tag="prod")
                            for j in range(npair):
                                p = g * npair + j
                                nc.vector.tensor_scalar_mul(
                                    prod[:, j * S:(j + 1) * S],
                                    ps2_sb[:],
                                    ptb2[:, p:p + 1])
                            actb = actpool.tile([128, npair * S], f32r,
                                                tag="actb")
                            nc.scalar.activation(actb[:], prod[:], AF_SILU)
                            for j in range(npair):
                                p = g * npair + j
                                last = (g == GRP - 1 and j == npair - 1)
                                for nh in range(2):
                                    nc.tensor.matmul(
                                        score_ps[:, nh * 512:(nh + 1) * 512],
                                        (big_sb[:, 63 - p: 63 - p + 128]),
                                        (actb[:, j * S + nh * 512:
                                                j * S + nh * 512 + 512]),
                                        start=False, stop=last)

                    out_sb = outpool.tile([128, S], f32, tag="out_sb")
                    nc.vector.tensor_copy(out_sb[:], score_ps[:])
                    nc.sync.dma_start(out=out[tb * 128:(tb + 1) * 128, :],
                                      in_=out_sb[:])

            if loop_n > 0:
                with tc.For_i(0, loop_n, 1,
                              hint_engines=(ET.Activation, ET.PE)):
                    emit_body()
            else:
                emit_body()
    nc.compile()
    return nc


def _get_nc(variant=VARIANT, loop_n=0):
    key = (variant, loop_n, _ACT_NAME)
    if key not in _compiled:
        _compiled[key] = _build_nc(variant=variant, loop_n=loop_n)
    return _compiled[key]


def make_in_maps(target_val, source_val, Wt, Ws, wt_out, ws_out, iw, bias_f):
    wtT = np.ascontiguousarray(Wt.T)                      # [D, R]
    wsT = np.ascontiguousarray(Ws.T)                      # [D, R]
    wsT2 = np.ascontiguousarray(np.concatenate([wsT, wsT], axis=1))  # [D,128]
    wtb_col = np.ascontiguousarray(
        np.concatenate([wt_out, np.ones(1, np.float32)])[:, None])  # [R+1,1]
    ws_rep256 = np.ascontiguousarray(
        np.broadcast_to(ws_out[:, None], (R, TBLK)))      # [R, 256]
    big = np.zeros((128, 192), dtype=np.float32)
    big[0:R, 63] = iw
    big[R:128, 127] = iw
    bias_row = np.full((1, TBLK), bias_f, dtype=np.float32)

    svT = [np.ascontiguousarray(source_val[b].T) for b in range(B)]

    in_maps = []
    for c in range(NCORES):
        b, ti = c // 4, c % 4
        in_maps.append({
            "tvT": np.ascontiguousarray(
                target_val[b, ti * TBLK:(ti + 1) * TBLK, :].T),
            "svT": svT[b],
            "wtT": wtT,
            "wsT2": wsT2,
            "wtb_col": wtb_col,
            "ws_rep256": ws_rep256,
            "big": big,
            "bias_row": bias_row,
            "ones_row": np.ones((1, S), dtype=np.float32),
        })
    return in_maps


def kernel(target_val, source_val, Wt, Ws, wt_out, ws_out,
           interaction_weight, bias):
    from concourse.bass_utils import run_bass_kernel_spmd

    target_val = np.asarray(target_val, dtype=np.float32)
    source_val = np.asarray(source_val, dtype=np.float32)
    Wt = np.asarray(Wt, dtype=np.float32)
    Ws = np.asarray(Ws, dtype=np.float32)
    wt_out = np.asarray(wt_out, dtype=np.float32)
    ws_out = np.asarray(ws_out, dtype=np.float32)
    iw = np.asarray(interaction_weight, dtype=np.float32)
    bias_f = float(np.asarray(bias, dtype=np.float32))

    nc = _get_nc()
    in_maps = make_in_maps(target_val, source_val, Wt, Ws, wt_out, ws_out,
                           iw, bias_f)
    res = run_bass_kernel_spmd(nc, in_maps, core_ids=list(range(NCORES)))

    scores = np.empty((B, T, S), dtype=np.float32)
    for c in range(NCORES):
        b, ti = c // 4, c % 4
        scores[b, ti * TBLK:(ti + 1) * TBLK, :] = res.results[c]["out"]
    return scores



# revision 2
# speedup vs baseline: 1.3534x; 1.3534x over previous
"""Trainium2 Bass kernel for AdditiveLowRankPairwise (v2: ACT+DVE split).

scores[b,t,s] = sum_r iw[r]*silu(pt[b,t,r]*ps[b,s,r]) + tl[b,t] + sl[b,s] + bias
  pt = target_val @ Wt.T   [B,T,R]
  ps = source_val @ Ws.T   [B,S,R]
  tl = pt @ wt_out         [B,T]
  sl = ps @ ws_out         [B,S]

B=2, T=S=1024, D=512, R=64.  8 cores: core c handles b=c//4, t-rows
[(c%4)*256, (c%4+1)*256).

Per core the 256 t-rows are processed as 2 blocks (tb) of 128 rows; each
block is 64 "pairs" p -> rows {p, 64+p} packed on the 128 partitions
(r duplicated on partition halves, like the baseline).  The silu work is
split across two engines:

  - ACT pairs (p < NA): one ACT instruction silu(ps2[q,s]*pt2[q,p]) via the
    per-partition scale operand, reading ps2 straight from PSUM, writing
    bf16 to SBUF.  (exact silu)
  - DVE pairs (p >= NA): 3 DVE passes in bf16 computing the fitted
    approximation  h(x) = DI*x*clamp(x,-W,W) + A*x + E  with
    x = pt*ps:  x = ps_bf*pt (tensor_scalar, 4x);  c = clamp(x)
    (2-op tensor_scalar, 4x);  y = (x*DI)*c  (scalar_tensor_tensor, 2x).
    The linear term A*x sums to a rank-64 bilinear form that is folded into
    the score-init matmul (stationary rows ws_out[r] + A*iw[r]*pt2[r,q] for
    DVE columns q), and E sums to a constant folded into the per-column
    bias row.  Fitted on the actual product distribution:
    rms(h-silu) = 0.033 -> end-to-end rel err ~2e-3 (gate 2e-2).

  - PE reduces every pair the same way: one-hot iw stationary (bf16 big
    matrix slice trick) against the bf16 activation tile, accumulated in
    the f32 score PSUM, 2 matmuls of N=512 per pair.  Score PSUM is
    initialized by a [65,128] matmul providing sl[s] + tl[t] + bias (+ the
    DVE fold terms).

ACT and DVE pairs are interleaved in program order so both engines stream
concurrently into the PE accumulation.

loop_n>0 wraps the body in an on-device For_i loop (wall-clock-delta
timing harness; see bench3.py).
"""

import numpy as np

B, T, S, D, R = 2, 1024, 1024, 512, 64
TBLK = 256          # t-rows per core
NCORES = 8
NA = 35             # ACT pairs per 128-row block (DVE pairs: 64-NA)
# fitted silu approximation h(x) = DI*x*clamp(x,-W,W) + A*x + E
W_C = 2.52936
DI = 0.18791
A_LIN = 0.49981
E_C = 0.01266
_ACT_NAME = "Silu"  # sim override: CoreSim lacks Silu; tests may set "Sigmoid"

_compiled = {}


def _pair_order(na):
    """Interleave ACT pairs (0..na-1) and DVE pairs (na..63) proportionally
    so both producers stream concurrently."""
    nd = 64 - na
    order = []
    ia = id_ = 0
    for j in range(64):
        # pick the stream that is behind its proportional share
        if ia * nd <= id_ * na and ia < na:
            order.append(ia)
            ia += 1
        elif id_ < nd:
            order.append(na + id_)
            id_ += 1
        else:
            order.append(ia)
            ia += 1
    return order


def _build_nc(na=NA, loop_n=0):
    import concourse.mybir as mybir
    import concourse.tile as tile
    from concourse import bacc

    f32 = mybir.dt.float32
    f32r = mybir.dt.float32r
    bf16 = mybir.dt.bfloat16
    AF = mybir.ActivationFunctionType
    AF_SILU = getattr(AF, _ACT_NAME)
    ET = mybir.EngineType
    OP = mybir.AluOpType

    nc = bacc.Bacc("TRN2", target_bir_lowering=False, debug=False)

    tvT = nc.dram_tensor("tvT", [D, TBLK], f32r, kind="ExternalInput")
    svT = nc.dram_tensor("svT", [D, S], f32r, kind="ExternalInput")
    wtT = nc.dram_tensor("wtT", [D, R], f32r, kind="ExternalInput")
    wsT2 = nc.dram_tensor("wsT2", [D, 128], f32r, kind="ExternalInput")
    wtb_col = nc.dram_tensor("wtb_col", [R + 1, 1], f32r, kind="ExternalInput")
    ws_rep256 = nc.dram_tensor("ws_rep256", [R, TBLK], f32r,
                               kind="ExternalInput")
    big = nc.dram_tensor("big", [128, 192], bf16, kind="ExternalInput")
    bias_row = nc.dram_tensor("bias_row", [1, TBLK], f32r,
                              kind="ExternalInput")
    ones_row = nc.dram_tensor("ones_row", [1, S], f32r, kind="ExternalInput")
    aiw = nc.dram_tensor("aiw", [R, 1], f32, kind="ExternalInput")
    out = nc.dram_tensor("out", [TBLK, S], f32, kind="ExternalOutput")

    order = _pair_order(na)

    with tile.TileContext(nc) as tc:
        with (
            tc.tile_pool(name="const", bufs=1) as cpool,
            tc.tile_pool(name="ptb", bufs=2) as ptbpool,
            tc.tile_pool(name="actb", bufs=4) as actpool,
            tc.tile_pool(name="xb", bufs=3) as xpool,
            tc.tile_pool(name="cb", bufs=3) as clpool,
            tc.tile_pool(name="yb", bufs=4) as ypool,
            tc.tile_pool(name="ps2_psum", bufs=1, space="PSUM") as ps2pool,
            tc.tile_pool(name="pt_psum", bufs=1, space="PSUM") as ptpool,
            tc.tile_pool(name="tl_psum", bufs=1, space="PSUM") as tlpool,
            tc.tile_pool(name="score_psum", bufs=2, space="PSUM") as spool,
            tc.tile_pool(name="outsb", bufs=2) as outpool,
        ):
            def emit_body():
                wtT_sb = cpool.tile([128, 4 * R], f32r, tag="wtT_sb")
                wsT2_sb = cpool.tile([128, 4 * 128], f32r, tag="wsT2_sb")
                wtb_sb = cpool.tile([R + 1, 1], f32r, tag="wtb_sb")
                slt_stat = cpool.tile([R + 1, TBLK], f32r, tag="slt_stat")
                big_sb = cpool.tile([128, 192], bf16, tag="big_sb")
                tv_sb = cpool.tile([128, 4 * TBLK], f32r, tag="tv_sb")
                sv_k = [cpool.tile([128, S], f32r, tag=f"sv_{k}",
                                   name=f"sv_{k}")
                        for k in range(4)]
                ps2_bf = cpool.tile([128, S], bf16, tag="ps2_bf")
                psl = cpool.tile([R + 1, S], f32r, tag="psl")
                pt_sb = cpool.tile([R + 1, TBLK], f32r, tag="pt_sb")
                aiw_sb = cpool.tile([R, 1], f32, tag="aiw_sb")

                for k in range(4):
                    nc.sync.dma_start(out=sv_k[k][:],
                                      in_=svT[k * 128:(k + 1) * 128, :])
                    nc.sync.dma_start(out=wtT_sb[:, k * R:(k + 1) * R],
                                      in_=wtT[k * 128:(k + 1) * 128, :])
                    nc.sync.dma_start(out=wsT2_sb[:, k * 128:(k + 1) * 128],
                                      in_=wsT2[k * 128:(k + 1) * 128, :])
                    nc.sync.dma_start(out=tv_sb[:, k * TBLK:(k + 1) * TBLK],
                                      in_=tvT[k * 128:(k + 1) * 128, :])
                nc.sync.dma_start(out=wtb_sb[:], in_=wtb_col[:])
                nc.sync.dma_start(out=slt_stat[0:R, :], in_=ws_rep256[:])
                nc.sync.dma_start(out=big_sb[:], in_=big[:])
                nc.sync.dma_start(out=pt_sb[R:R + 1, :], in_=bias_row[:])
                nc.sync.dma_start(out=aiw_sb[:], in_=aiw[:])

                # ---- projections on PE (float32r, full rate) ----
                ps2 = ps2pool.tile([128, S], f32, tag="ps2")
                for kc in range(4):
                    for nh in range(2):
                        nc.tensor.matmul(
                            ps2[:, nh * 512:(nh + 1) * 512],
                            (wsT2_sb[:, kc * 128:(kc + 1) * 128]),
                            (sv_k[kc][:, nh * 512:(nh + 1) * 512]),
                            start=(kc == 0), stop=(kc == 3))
                pt_ps = ptpool.tile([R, TBLK], f32, tag="pt_ps")
                for kc in range(4):
                    nc.tensor.matmul(
                        pt_ps[:],
                        (wtT_sb[:, kc * R:(kc + 1) * R]),
                        (tv_sb[:, kc * TBLK:(kc + 1) * TBLK]),
                        start=(kc == 0), stop=(kc == 3))
                # SBUF copies: psl rows 0:64 = ps (f32) + ones row 64 for the
                # psum-init matmul; bf16 copy of the duplicated ps2 for the
                # DVE path; pt.
                nc.vector.tensor_copy(ps2_bf[:], ps2[:])
                nc.vector.tensor_copy(psl[0:R, :], ps2[0:R, :])
                nc.sync.dma_start(out=psl[R:R + 1, :], in_=ones_row[:])
                nc.vector.tensor_copy(pt_sb[0:R, :], pt_ps[:])

                # tl+bias row: one matmul over [65,(pt;bias_row)] -> [1, 256]
                tl_ps = tlpool.tile([1, TBLK], f32, tag="tl_ps")
                nc.tensor.matmul(tl_ps[:], (wtb_sb[:]), (pt_sb[:]),
                                 start=True, stop=True)
                nc.vector.tensor_copy(slt_stat[R:R + 1, :], tl_ps[:])

                # fold A*iw[r]*pt2[r,q] into the init stationary for DVE cols
                if na < 64:
                    for tb in range(2):
                        for half in range(2):
                            c0 = tb * 128 + half * 64 + na
                            c1 = tb * 128 + half * 64 + 64
                            nc.vector.scalar_tensor_tensor(
                                slt_stat[0:R, c0:c1],
                                pt_sb[0:R, c0:c1],
                                aiw_sb[:, 0:1],
                                slt_stat[0:R, c0:c1],
                                OP.mult, OP.add)

                for tb in range(2):
                    ptb2 = ptbpool.tile([128, R], f32, tag="ptb2")
                    nc.vector.tensor_copy(ptb2[0:R, :],
                                          pt_sb[0:R, tb * 128: tb * 128 + R])
                    nc.vector.tensor_copy(
                        ptb2[R:128, :],
                        pt_sb[0:R, tb * 128 + R: tb * 128 + 128])

                    score_ps = spool.tile([128, S], f32, tag="score_ps")
                    # init psum with sl[s] + tl[t] + bias (+ DVE folds)
                    for nh in range(2):
                        nc.tensor.matmul(
                            score_ps[:, nh * 512:(nh + 1) * 512],
                            (slt_stat[:, tb * 128:(tb + 1) * 128]),
                            (psl[:, nh * 512: nh * 512 + 512]),
                            start=True, stop=False)

                    for j, p in enumerate(order):
                        if p < na:
                            buf = actpool.tile([128, S], bf16, tag="actb")
                            nc.scalar.activation(buf[:], ps2[:], AF_SILU,
                                                 scale=ptb2[:, p:p + 1])
                        else:
                            xt = xpool.tile([128, S], bf16, tag="xb")
                            nc.vector.tensor_scalar_mul(
                                xt[:], ps2_bf[:], ptb2[:, p:p + 1])
                            ct = clpool.tile([128, S], bf16, tag="cb")
                            nc.vector.tensor_scalar(
                                ct[:], xt[:], -W_C, W_C, OP.max, OP.min)
                            buf = ypool.tile([128, S], bf16, tag="yb")
                            nc.vector.scalar_tensor_tensor(
                                buf[:], xt[:], DI, ct[:], OP.mult, OP.mult)
                        last = (j == 63)
                        for nh in range(2):
                            nc.tensor.matmul(
                                score_ps[:, nh * 512:(nh + 1) * 512],
                                (big_sb[:, 63 - p: 63 - p + 128]),
                                (buf[:, nh * 512: nh * 512 + 512]),
                                start=False, stop=(last and nh == 1))

                    out_sb = outpool.tile([128, S], f32, tag="out_sb")
                    if tb == 0:
                        nc.scalar.copy(out_sb[:], score_ps[:])
                    else:
                        nc.vector.tensor_copy(out_sb[:], score_ps[:])
                    nc.sync.dma_start(out=out[tb * 128:(tb + 1) * 128, :],
                                      in_=out_sb[:])

            if loop_n > 0:
                with tc.For_i(0, loop_n, 1,
                              hint_engines=(ET.Activation, ET.PE, ET.DVE)):
                    emit_body()
            else:
                emit_body()
    nc.compile()
    return nc


def _get_nc(na=NA, loop_n=0):
    key = (na, loop_n, _ACT_NAME)
    if key not in _compiled:
        _compiled[key] = _build_nc(na=na, loop_n=loop_n)
    return _compiled[key]


def make_in_maps(target_val, source_val, Wt, Ws, wt_out, ws_out, iw, bias_f,
                 na=NA):
    import ml_dtypes
    bf16 = ml_dtypes.bfloat16

    wtT = np.ascontiguousarray(Wt.T)                      # [D, R]
    wsT = np.ascontiguousarray(Ws.T)                      # [D, R]
    wsT2 = np.ascontiguousarray(np.concatenate([wsT, wsT], axis=1))  # [D,128]
    wtb_col = np.ascontiguousarray(
        np.concatenate([wt_out, np.ones(1, np.float32)])[:, None])  # [R+1,1]
    ws_rep256 = np.ascontiguousarray(
        np.broadcast_to(ws_out[:, None], (R, TBLK))).astype(np.float32)
    big = np.zeros((128, 192), dtype=bf16)
    big[0:R, 63] = iw.astype(bf16)
    big[R:128, 127] = iw.astype(bf16)
    # per-column bias: +E_C*sum(iw) for DVE columns
    bias_row = np.full((1, TBLK), bias_f, dtype=np.float32)
    esum = float(E_C * iw.sum())
    for tb in range(2):
        for half in range(2):
            c0 = tb * 128 + half * 64 + na
            c1 = tb * 128 + half * 64 + 64
            bias_row[0, c0:c1] += esum
    aiw = np.ascontiguousarray((A_LIN * iw)[:, None]).astype(np.float32)

    svT = [np.ascontiguousarray(source_val[b].T) for b in range(B)]

    in_maps = []
    for c in range(NCORES):
        b, ti = c // 4, c % 4
        in_maps.append({
            "tvT": np.ascontiguousarray(
                target_val[b, ti * TBLK:(ti + 1) * TBLK, :].T),
            "svT": svT[b],
            "wtT": wtT,
            "wsT2": wsT2,
            "wtb_col": wtb_col,
            "ws_rep256": ws_rep256,
            "big": big,
            "bias_row": bias_row,
            "ones_row": np.ones((1, S), dtype=np.float32),
            "aiw": aiw,
        })
    return in_maps


def kernel(target_val, source_val, Wt, Ws, wt_out, ws_out,
           interaction_weight, bias):
    from concourse.bass_utils import run_bass_kernel_spmd

    target_val = np.asarray(target_val, dtype=np.float32)
    source_val = np.asarray(source_val, dtype=np.float32)
    Wt = np.asarray(Wt, dtype=np.float32)
    Ws = np.asarray(Ws, dtype=np.float32)
    wt_out = np.asarray(wt_out, dtype=np.float32)
    ws_out = np.asarray(ws_out, dtype=np.float32)
    iw = np.asarray(interaction_weight, dtype=np.float32)
    bias_f = float(np.asarray(bias, dtype=np.float32))

    nc = _get_nc()
    in_maps = make_in_maps(target_val, source_val, Wt, Ws, wt_out, ws_out,
                           iw, bias_f)
    res = run_bass_kernel_spmd(nc, in_maps, core_ids=list(range(NCORES)))

    scores = np.empty((B, T, S), dtype=np.float32)
    for c in range(NCORES):
        b, ti = c // 4, c % 4
        scores[b, ti * TBLK:(ti + 1) * TBLK, :] = res.results[c]["out"]
    return scores


# revision 8
# speedup vs baseline: 2.2784x; 1.6835x over previous
"""Trainium2 Bass kernel for AdditiveLowRankPairwise (v2: ACT+DVE split).

scores[b,t,s] = sum_r iw[r]*silu(pt[b,t,r]*ps[b,s,r]) + tl[b,t] + sl[b,s] + bias
  pt = target_val @ Wt.T   [B,T,R]
  ps = source_val @ Ws.T   [B,S,R]
  tl = pt @ wt_out         [B,T]
  sl = ps @ ws_out         [B,S]

B=2, T=S=1024, D=512, R=64.  8 cores: core c handles b=c//4, t-rows
[(c%4)*256, (c%4+1)*256).

Per core the 256 t-rows are processed as 2 blocks (tb) of 128 rows; each
block is 64 "pairs" p -> rows {p, 64+p} packed on the 128 partitions
(r duplicated on partition halves, like the baseline).  The silu work is
split across two engines:

  - ACT pairs (p < NA): one ACT instruction silu(ps2[q,s]*pt2[q,p]) via the
    per-partition scale operand, reading ps2 straight from PSUM, writing
    bf16 to SBUF.  (exact silu)
  - DVE pairs (p >= NA): 2 DVE passes in bf16 computing the fitted
    approximation  h(x) = DI*x*clamp(x,-W,W) + A*x + E  with x = pt*ps,
    using the identity x*clamp(x,-W,W) = |x|*min(|x|,W):
      pass1: x1 = |ps|_bf*|pt|    (tensor_scalar mult; 4x)
      pass2: c  = min(x1,W)       (tensor_scalar min; 4x)
      pass3: y  = c*x1            (tensor_tensor mult; 2x)
    (|ps| precomputed once per core, |pt| once per block)
    The DI factor is folded into a second one-hot stationary (bigD = iw*DI).
    The linear term A*x sums to a rank-64 bilinear form that is folded into
    the score-init matmul (stationary rows ws_out[r] + A*iw[r]*pt2[r,q] for
    DVE columns q), and E sums to a constant folded into the per-column
    bias row.  Fitted on the actual product distribution:
    rms(h-silu) = 0.033 -> end-to-end rel err ~2e-3 (gate 2e-2).

  - PE reduces every pair the same way: one-hot iw stationary (bf16 big
    matrix slice trick) against the bf16 activation tile, accumulated in
    the f32 score PSUM, 2 matmuls of N=512 per pair.  Score PSUM is
    initialized by a [65,128] matmul providing sl[s] + tl[t] + bias (+ the
    DVE fold terms).

ACT and DVE pairs are interleaved in program order so both engines stream
concurrently into the PE accumulation.

loop_n>0 wraps the body in an on-device For_i loop (wall-clock-delta
timing harness; see bench3.py).
"""

import numpy as np

B, T, S, D, R = 2, 1024, 1024, 512, 64
TBLK = 256          # t-rows per core
NCORES = 8
NA = 33             # ACT pairs per 128-row block (DVE pairs: 64-NA)
# fitted silu approximation h(x) = DI*x*clamp(x,-W,W) + A*x + E
W_C = 2.52936
DI = 0.18791
A_LIN = 0.49981
E_C = 0.01266
_ACT_NAME = "Silu"  # sim override: CoreSim lacks Silu; tests may set "Sigmoid"

_compiled = {}


def _pair_order(na):
    """Interleave ACT pairs (0..na-1) and DVE pairs (na..63) proportionally
    so both producers stream concurrently."""
    nd = 64 - na
    order = []
    ia = id_ = 0
    for j in range(64):
        # pick the stream that is behind its proportional share
        if ia * nd <= id_ * na and ia < na:
            order.append(ia)
            ia += 1
        elif id_ < nd:
            order.append(na + id_)
            id_ += 1
        else:
            order.append(ia)
            ia += 1
    return order


def _build_nc(na=NA, loop_n=0):
    import concourse.mybir as mybir
    import concourse.tile as tile
    from concourse import bacc

    f32 = mybir.dt.float32
    f32r = mybir.dt.float32r
    bf16 = mybir.dt.bfloat16
    AF = mybir.ActivationFunctionType
    AF_SILU = getattr(AF, _ACT_NAME)
    ET = mybir.EngineType
    OP = mybir.AluOpType

    nc = bacc.Bacc("TRN2", target_bir_lowering=False, debug=False)

    tvT = nc.dram_tensor("tvT", [D, TBLK], f32r, kind="ExternalInput")
    svT = nc.dram_tensor("svT", [D, S], f32r, kind="ExternalInput")
    wtT = nc.dram_tensor("wtT", [D, R], f32r, kind="ExternalInput")
    wsT2 = nc.dram_tensor("wsT2", [D, 128], f32r, kind="ExternalInput")
    wtb_col = nc.dram_tensor("wtb_col", [R + 1, 1], f32r, kind="ExternalInput")
    ws_rep256 = nc.dram_tensor("ws_rep256", [R, TBLK], f32r,
                               kind="ExternalInput")
    big = nc.dram_tensor("big", [128, 192], bf16, kind="ExternalInput")
    bigd = nc.dram_tensor("bigd", [128, 192], bf16, kind="ExternalInput")
    bias_row = nc.dram_tensor("bias_row", [1, TBLK], f32r,
                              kind="ExternalInput")
    ones_row = nc.dram_tensor("ones_row", [1, S], f32r, kind="ExternalInput")
    aiw = nc.dram_tensor("aiw", [R, 1], f32, kind="ExternalInput")
    out = nc.dram_tensor("out", [TBLK, S], f32, kind="ExternalOutput")

    order = _pair_order(na)

    with tile.TileContext(nc) as tc:
        with (
            tc.tile_pool(name="const", bufs=1) as cpool,
            tc.tile_pool(name="ptb", bufs=2) as ptbpool,
            tc.tile_pool(name="actb", bufs=8) as actpool,
            tc.tile_pool(name="xb", bufs=4) as xpool,
            tc.tile_pool(name="cb", bufs=4) as clpool,
            tc.tile_pool(name="yb", bufs=8) as ypool,
            tc.tile_pool(name="ps2_psum", bufs=1, space="PSUM") as ps2pool,
            tc.tile_pool(name="pt_psum", bufs=1, space="PSUM") as ptpool,
            tc.tile_pool(name="tl_psum", bufs=1, space="PSUM") as tlpool,
            tc.tile_pool(name="score_psum", bufs=2, space="PSUM") as spool,
            tc.tile_pool(name="outsb", bufs=2) as outpool,
        ):
            def emit_body():
                wtT_sb = cpool.tile([128, 4 * R], f32r, tag="wtT_sb")
                wsT2_sb = cpool.tile([128, 4 * 128], f32r, tag="wsT2_sb")
                wtb_sb = cpool.tile([R + 1, 1], f32r, tag="wtb_sb")
                slt_stat = cpool.tile([R + 1, TBLK], f32r, tag="slt_stat")
                big_sb = cpool.tile([128, 192], bf16, tag="big_sb")
                bigd_sb = cpool.tile([128, 192], bf16, tag="bigd_sb")
                tv_sb = cpool.tile([128, 4 * TBLK], f32r, tag="tv_sb")
                sv_k = [cpool.tile([128, S], f32r, tag=f"sv_{k}",
                                   name=f"sv_{k}")
                        for k in range(4)]
                ps2_bf = cpool.tile([128, S], bf16, tag="ps2_bf")
                psl = cpool.tile([R + 1, S], f32r, tag="psl")
                pt_sb = cpool.tile([R + 1, TBLK], f32r, tag="pt_sb")
                aiw_sb = cpool.tile([R, 1], f32, tag="aiw_sb")

                # Preload the Silu activation table during the DMA prologue:
                # a tiny dummy activation forces the ACT_TABLE_LOAD early.
                warm = cpool.tile([1, 2], f32, tag="warm")
                nc.vector.memset(warm[:], 0.0)
                nc.scalar.activation(warm[:], warm[:], AF_SILU)

                for k in range(4):
                    nc.sync.dma_start(out=sv_k[k][:],
                                      in_=svT[k * 128:(k + 1) * 128, :])
                    nc.sync.dma_start(out=wtT_sb[:, k * R:(k + 1) * R],
                                      in_=wtT[k * 128:(k + 1) * 128, :])
                    nc.sync.dma_start(out=wsT2_sb[:, k * 128:(k + 1) * 128],
                                      in_=wsT2[k * 128:(k + 1) * 128, :])
                    nc.sync.dma_start(out=tv_sb[:, k * TBLK:(k + 1) * TBLK],
                                      in_=tvT[k * 128:(k + 1) * 128, :])
                nc.sync.dma_start(out=wtb_sb[:], in_=wtb_col[:])
                nc.sync.dma_start(out=slt_stat[0:R, :], in_=ws_rep256[:])
                nc.sync.dma_start(out=big_sb[:], in_=big[:])
                nc.sync.dma_start(out=bigd_sb[:], in_=bigd[:])
                nc.sync.dma_start(out=pt_sb[R:R + 1, :], in_=bias_row[:])
                nc.sync.dma_start(out=aiw_sb[:], in_=aiw[:])

                # ---- projections on PE (float32r, full rate) ----
                ps2 = ps2pool.tile([128, S], f32, tag="ps2")
                for kc in range(4):
                    for nh in range(2):
                        nc.tensor.matmul(
                            ps2[:, nh * 512:(nh + 1) * 512],
                            (wsT2_sb[:, kc * 128:(kc + 1) * 128]),
                            (sv_k[kc][:, nh * 512:(nh + 1) * 512]),
                            start=(kc == 0), stop=(kc == 3))
                pt_ps = ptpool.tile([R, TBLK], f32, tag="pt_ps")
                for kc in range(4):
                    nc.tensor.matmul(
                        pt_ps[:],
                        (wtT_sb[:, kc * R:(kc + 1) * R]),
                        (tv_sb[:, kc * TBLK:(kc + 1) * TBLK]),
                        start=(kc == 0), stop=(kc == 3))
                # SBUF copies: psl rows 0:64 = ps (f32) + ones row 64 for the
                # psum-init matmul; bf16 copy of the duplicated ps2 for the
                # DVE path; pt.
                nc.vector.tensor_copy(ps2_bf[:], ps2[:])
                psa = cpool.tile([128, S], bf16, tag="psa")
                nc.vector.scalar_tensor_tensor(
                    psa[:], ps2_bf[:], -1.0, ps2_bf[:], OP.mult, OP.max)
                nc.vector.tensor_copy(psl[0:R, :], ps2[0:R, :])
                nc.sync.dma_start(out=psl[R:R + 1, :], in_=ones_row[:])
                nc.vector.tensor_copy(pt_sb[0:R, :], pt_ps[:])

                # tl+bias row: one matmul over [65,(pt;bias_row)] -> [1, 256]
                tl_ps = tlpool.tile([1, TBLK], f32, tag="tl_ps")
                nc.tensor.matmul(tl_ps[:], (wtb_sb[:]), (pt_sb[:]),
                                 start=True, stop=True)
                nc.vector.tensor_copy(slt_stat[R:R + 1, :], tl_ps[:])

                # fold A*iw[r]*pt2[r,q] into the init stationary for DVE cols
                if na < 64:
                    for tb in range(2):
                        for half in range(2):
                            c0 = tb * 128 + half * 64 + na
                            c1 = tb * 128 + half * 64 + 64
                            nc.vector.scalar_tensor_tensor(
                                slt_stat[0:R, c0:c1],
                                pt_sb[0:R, c0:c1],
                                aiw_sb[:, 0:1],
                                slt_stat[0:R, c0:c1],
                                OP.mult, OP.add)

                for tb in range(2):
                    ptb2 = ptbpool.tile([128, R], f32, tag="ptb2")
                    nc.vector.tensor_copy(ptb2[0:R, :],
                                          pt_sb[0:R, tb * 128: tb * 128 + R])
                    nc.vector.tensor_copy(
                        ptb2[R:128, :],
                        pt_sb[0:R, tb * 128 + R: tb * 128 + 128])
                    ptb2a = ptbpool.tile([128, R], f32, tag="ptb2a")
                    nc.vector.scalar_tensor_tensor(
                        ptb2a[:], ptb2[:], -1.0, ptb2[:], OP.mult, OP.max)

                    score_ps = spool.tile([128, S], f32, tag="score_ps")
                    # init psum with sl[s] + tl[t] + bias (+ DVE folds)
                    for nh in range(2):
                        nc.tensor.matmul(
                            score_ps[:, nh * 512:(nh + 1) * 512],
                            (slt_stat[:, tb * 128:(tb + 1) * 128]),
                            (psl[:, nh * 512: nh * 512 + 512]),
                            start=True, stop=False)

                    for j, p in enumerate(order):
                        if p < na:
                            buf = actpool.tile([128, S], bf16, tag="actb")
                            nc.scalar.activation(buf[:], ps2[:], AF_SILU,
                                                 scale=ptb2[:, p:p + 1])
                            stat = big_sb
                        else:
                            xt = xpool.tile([128, S], bf16, tag="xb")
                            nc.vector.tensor_scalar_mul(
                                xt[:], psa[:], ptb2a[:, p:p + 1])
                            ct = clpool.tile([128, S], bf16, tag="cb")
                            nc.vector.tensor_scalar_min(
                                ct[:], xt[:], W_C)
                            buf = ypool.tile([128, S], bf16, tag="yb")
                            nc.vector.tensor_tensor(
                                buf[:], ct[:], xt[:], OP.mult)
                            stat = bigd_sb
                        last = (j == 63)
                        for nh in range(2):
                            nc.tensor.matmul(
                                score_ps[:, nh * 512:(nh + 1) * 512],
                                (stat[:, 63 - p: 63 - p + 128]),
                                (buf[:, nh * 512: nh * 512 + 512]),
                                start=False, stop=(last and nh == 1))

                    out_sb = outpool.tile([128, S], f32, tag="out_sb")
                    if tb == 0:
                        nc.scalar.copy(out_sb[:], score_ps[:])
                    else:
                        nc.vector.tensor_copy(out_sb[:], score_ps[:])
                    nc.sync.dma_start(out=out[tb * 128:(tb + 1) * 128, :],
                                      in_=out_sb[:])

            if loop_n > 0:
                with tc.For_i(0, loop_n, 1,
                              hint_engines=(ET.Activation, ET.PE, ET.DVE)):
                    emit_body()
            else:
                emit_body()
    nc.compile()
    return nc


def _get_nc(na=NA, loop_n=0):
    key = (na, loop_n, _ACT_NAME)
    if key not in _compiled:
        _compiled[key] = _build_nc(na=na, loop_n=loop_n)
    return _compiled[key]


def make_in_maps(target_val, source_val, Wt, Ws, wt_out, ws_out, iw, bias_f,
                 na=NA):
    import ml_dtypes
    bf16 = ml_dtypes.bfloat16

    wtT = np.ascontiguousarray(Wt.T)                      # [D, R]
    wsT = np.ascontiguousarray(Ws.T)                      # [D, R]
    wsT2 = np.ascontiguousarray(np.concatenate([wsT, wsT], axis=1))  # [D,128]
    wtb_col = np.ascontiguousarray(
        np.concatenate([wt_out, np.ones(1, np.float32)])[:, None])  # [R+1,1]
    ws_rep256 = np.ascontiguousarray(
        np.broadcast_to(ws_out[:, None], (R, TBLK))).astype(np.float32)
    big = np.zeros((128, 192), dtype=bf16)
    big[0:R, 63] = iw.astype(bf16)
    big[R:128, 127] = iw.astype(bf16)
    bigd = np.zeros((128, 192), dtype=bf16)
    bigd[0:R, 63] = (iw * DI).astype(bf16)
    bigd[R:128, 127] = (iw * DI).astype(bf16)
    # per-column bias: +E_C*sum(iw) for DVE columns
    bias_row = np.full((1, TBLK), bias_f, dtype=np.float32)
    esum = float(E_C * iw.sum())
    for tb in range(2):
        for half in range(2):
            c0 = tb * 128 + half * 64 + na
            c1 = tb * 128 + half * 64 + 64
            bias_row[0, c0:c1] += esum
    aiw = np.ascontiguousarray((A_LIN * iw)[:, None]).astype(np.float32)

    svT = [np.ascontiguousarray(source_val[b].T) for b in range(B)]

    in_maps = []
    for c in range(NCORES):
        b, ti = c // 4, c % 4
        in_maps.append({
            "tvT": np.ascontiguousarray(
                target_val[b, ti * TBLK:(ti + 1) * TBLK, :].T),
            "svT": svT[b],
            "wtT": wtT,
            "wsT2": wsT2,
            "wtb_col": wtb_col,
            "ws_rep256": ws_rep256,
            "big": big,
            "bigd": bigd,
            "bias_row": bias_row,
            "ones_row": np.ones((1, S), dtype=np.float32),
            "aiw": aiw,
        })
    return in_maps


def kernel(target_val, source_val, Wt, Ws, wt_out, ws_out,
           interaction_weight, bias):
    from concourse.bass_utils import run_bass_kernel_spmd

    target_val = np.asarray(target_val, dtype=np.float32)
    source_val = np.asarray(source_val, dtype=np.float32)
    Wt = np.asarray(Wt, dtype=np.float32)
    Ws = np.asarray(Ws, dtype=np.float32)
    wt_out = np.asarray(wt_out, dtype=np.float32)
    ws_out = np.asarray(ws_out, dtype=np.float32)
    iw = np.asarray(interaction_weight, dtype=np.float32)
    bias_f = float(np.asarray(bias, dtype=np.float32))

    nc = _get_nc()
    in_maps = make_in_maps(target_val, source_val, Wt, Ws, wt_out, ws_out,
                           iw, bias_f)
    res = run_bass_kernel_spmd(nc, in_maps, core_ids=list(range(NCORES)))

    scores = np.empty((B, T, S), dtype=np.float32)
    for c in range(NCORES):
        b, ti = c // 4, c % 4
        scores[b, ti * TBLK:(ti + 1) * TBLK, :] = res.results[c]["out"]
    return scores
